# revision 15
# baseline (speedup 1.0000x reference)
"""nn_HAN_Agg on 8 Trainium2 NeuronCores via Bass/Tile.

Sharding: core c = (relation r = c//2, dst-half = c%2).
Launch 1: per-relation GATConv aggregation. hs_pack = x_src[r] @ [W_src |
fold(att_src)] written to a DRAM table with 768B rows [hs(256) | al_s(8) |
pad]; al_d likewise in a 256B-row table.  Per 128-dst block, edges (sorted
by dst block, padded to multiples of 128, uniform chunk counts across
cores for SPMD) are processed as: dma_gather of hs+al_s rows and al_d
rows, ex = exp(leaky(al_s+al_d)), one-hot S^T = (dstloc == iota), then
TensorE matmuls accumulate num = S^T.T @ (ex*hs) and den = S^T.T @ ex in
PSUM.  agg = relu(num/den + bias).
Launch 2 (after a host reshuffle): semantic attention + linear head + L2
normalize, 2500 nodes per core.
"""
import os, sys
sys.path.insert(0, "/opt/trn_rl_repo")
import numpy as np
import ml_dtypes

BF16 = ml_dtypes.bfloat16

N, D, H, E = 20000, 256, 8, 320000
C = D // H
GAT_SLOPE = 0.2
SEM_SLOPE = 0.01
N_CORES = 8
HALF = N // 2                    # dst nodes per core (10000)
ROWW = 384                       # packed hs row: [hs 256 | al_s 8 | pad] bf16 = 768B

_cache = {}
LAST_RUN_INFO = {}


def _ceil(a, b):
    return (a + b - 1) // b


def _build_launch1(ch_per_block, group_splits, tot_ch):
    from concourse import mybir, bacc
    from concourse.tile import TileContext
    dt = mybir.dt
    nblk = len(ch_per_block)
    n_e16 = tot_ch * 8           # idx cols (tot_ch*128/16)

    nc = bacc.Bacc("TRN2", target_bir_lowering=False, debug=False,
                   num_devices=N_CORES)

    xsT = nc.dram_tensor("xsT", [D, N], dt.bfloat16, kind="ExternalInput").ap()
    xnT = nc.dram_tensor("xnT", [D, HALF], dt.bfloat16, kind="ExternalInput").ap()
    wsp = nc.dram_tensor("wsp", [D, D + 8], dt.bfloat16, kind="ExternalInput").ap()
    wdf = nc.dram_tensor("wdf", [D, 8], dt.bfloat16, kind="ExternalInput").ap()
    bias_rep = nc.dram_tensor("bias_rep", [128, D], dt.float32, kind="ExternalInput").ap()
    gsrc = nc.dram_tensor("gsrc", [128, n_e16], dt.int16, kind="ExternalInput").ap()
    gdst = nc.dram_tensor("gdst", [128, n_e16], dt.int16, kind="ExternalInput").ap()
    dstloc = nc.dram_tensor("dstloc", [128, tot_ch], dt.float32, kind="ExternalInput").ap()

    hs_t = nc.dram_tensor("hs_t", [N + 8, ROWW], dt.bfloat16)
    ald_t = nc.dram_tensor("ald_t", [HALF + 8, 128], dt.bfloat16)

    agg = nc.dram_tensor("agg", [HALF, D], dt.bfloat16, kind="ExternalOutput").ap()

    nblk_a = _ceil(N, 128)       # 157 (last = 32 rows)

    with TileContext(nc) as tc:
        with tc.tile_pool(name="c1", bufs=1) as cp, \
             tc.tile_pool(name="p1", bufs=3) as p1, \
             tc.tile_pool(name="ps1", bufs=3, space="PSUM") as ps1:

            wsp_s = cp.tile([128, 2, D + 8], dt.bfloat16)
            wdf_s = cp.tile([128, 2, 8], dt.bfloat16)
            for cc in range(2):
                nc.sync.dma_start(out=wsp_s[:, cc, :], in_=wsp[cc * 128:(cc + 1) * 128, :])
                nc.sync.dma_start(out=wdf_s[:, cc, :], in_=wdf[cc * 128:(cc + 1) * 128, :])

            # dummy rows: hs_t[N] = [0.. | -1e9 x8 | 0..], ald_t[HALF] = 0
            drow = cp.tile([1, ROWW], dt.bfloat16)
            nc.vector.memset(drow[:], 0.0)
            nc.vector.memset(drow[:, D:D + 8], -1e9)
            nc.sync.dma_start(out=hs_t[N, :].unsqueeze(0), in_=drow[:])
            zrow = cp.tile([1, 128], dt.bfloat16)
            nc.vector.memset(zrow[:], 0.0)
            nc.sync.dma_start(out=ald_t[HALF, :].unsqueeze(0), in_=zrow[:])

            # phase 1a: hs_pack for all N rows
            for b4 in range(0, nblk_a, 4):
                hi = min(N, b4 * 128 + 512)
                w = hi - b4 * 128
                lhs = p1.tile([128, 2, 512], dt.bfloat16, tag="lhs")
                for cc in range(2):
                    nc.sync.dma_start(out=lhs[:, cc, :w],
                                      in_=xsT[cc * 128:(cc + 1) * 128, b4 * 128:hi])
                for j in range(_ceil(w, 128)):
                    blk = b4 + j
                    pb = min(128, N - blk * 128)
                    ps = ps1.tile([128, D + 8], dt.float32, tag="psa")
                    for cc in range(2):
                        nc.tensor.matmul(
                            ps[:pb],
                            lhsT=lhs[:, cc, j * 128:j * 128 + pb],
                            rhs=wsp_s[:, cc, :],
                            start=(cc == 0), stop=(cc == 1))
                    hsb = p1.tile([128, D + 8], dt.bfloat16, tag="hsb")
                    nc.scalar.activation(hsb[:pb], ps[:pb],
                                         mybir.ActivationFunctionType.Copy)
                    nc.sync.dma_start(
                        out=hs_t[blk * 128:blk * 128 + pb, 0:D + 8], in_=hsb[:pb])

            # phase 1b: al_d for owned half
            for b4 in range(0, nblk, 4):
                hi = min(HALF, b4 * 128 + 512)
                w = hi - b4 * 128
                lhd = p1.tile([128, 2, 512], dt.bfloat16, tag="lhd")
                for cc in range(2):
                    nc.sync.dma_start(out=lhd[:, cc, :w],
                                      in_=xnT[cc * 128:(cc + 1) * 128, b4 * 128:hi])
                for j in range(_ceil(w, 128)):
                    blk = b4 + j
                    pb = min(128, HALF - blk * 128)
                    psd = ps1.tile([128, 8], dt.float32, tag="psd")
                    for cc in range(2):
                        nc.tensor.matmul(
                            psd[:pb],
                            lhsT=lhd[:, cc, j * 128:j * 128 + pb],
                            rhs=wdf_s[:, cc, :],
                            start=(cc == 0), stop=(cc == 1))
                    aldb = p1.tile([128, 8], dt.bfloat16, tag="aldb")
                    nc.vector.tensor_copy(aldb[:pb], psd[:pb])
                    nc.sync.dma_start(
                        out=ald_t[blk * 128:blk * 128 + pb, 0:8], in_=aldb[:pb])

        # phase 2: edge aggregation
        with tc.tile_pool(name="c2", bufs=1) as cp2, \
             tc.tile_pool(name="p2", bufs=2) as p2, \
             tc.tile_pool(name="ps2", bufs=3, space="PSUM") as ps2:

            iota2 = cp2.tile([128, 128], dt.float32)
            nc.gpsimd.iota(iota2[:], pattern=[[1, 128]], channel_multiplier=0,
                           allow_small_or_imprecise_dtypes=True)
            bias2 = cp2.tile([128, D], dt.float32)
            nc.sync.dma_start(out=bias2[:], in_=bias_rep[:])

            ch_off = [0]
            for b in range(nblk):
                ch_off.append(ch_off[-1] + ch_per_block[b])

            _maxg = int(os.environ.get("L1_MAX_GROUPS", "9999"))
            for (gb0, gb1) in group_splits[:_maxg]:
                c0, c1 = ch_off[gb0], ch_off[gb1]
                gch = c1 - c0
                ne = gch * 128

                gsrc_g = p2.tile([128, gch * 8], dt.int16, tag="gsrc_g")
                nc.sync.dma_start(out=gsrc_g[:], in_=gsrc[:, c0 * 8:c1 * 8])
                gdst_g = p2.tile([128, gch * 8], dt.int16, tag="gdst_g")
                nc.sync.dma_start(out=gdst_g[:], in_=gdst[:, c0 * 8:c1 * 8])
                dl_g = p2.tile([128, gch], dt.float32, tag="dl_g")
                nc.sync.dma_start(out=dl_g[:], in_=dstloc[:, c0:c1])

                _og = os.environ.get("L1_ONLY_G", "")
                ghs = p2.tile([128, gch, ROWW], dt.bfloat16, tag="ghs")
                if _og in ("", "hs"):
                    nc.gpsimd.dma_gather(
                        out_ap=ghs[:], in_ap=hs_t[:], idxs_ap=gsrc_g[:],
                        num_idxs=ne, num_idxs_reg=ne, elem_size=ROWW, queue_num=0,
                        single_packet=False)
                gad = p2.tile([128, gch, 128], dt.bfloat16, tag="gad")
                if _og in ("", "ad"):
                    nc.gpsimd.dma_gather(
                        out_ap=gad[:], in_ap=ald_t[:], idxs_ap=gdst_g[:],
                        num_idxs=ne, num_idxs_reg=ne, elem_size=128, queue_num=0,
                        single_packet=False)

                if os.environ.get("L1_GATHER_ONLY"):
                    continue
                # ex = exp(leaky(al_s + al_d))
                ex = p2.tile([128, gch, 8], dt.float32, tag="ex")
                nc.vector.tensor_tensor(out=ex[:], in0=ghs[:, :, D:D + 8],
                                        in1=gad[:, :, 0:8], op=mybir.AluOpType.add)
                nc.vector.scalar_tensor_tensor(
                    out=ex[:], in0=ex[:], scalar=float(GAT_SLOPE), in1=ex[:],
                    op0=mybir.AluOpType.mult, op1=mybir.AluOpType.max)
                nc.scalar.activation(ex[:], ex[:], mybir.ActivationFunctionType.Exp)
                exb = p2.tile([128, gch, 8], dt.bfloat16, tag="exb")
                nc.vector.tensor_copy(exb[:], ex[:])

                # S^T one-hot (dstloc == iota)
                st = p2.tile([128, gch, 128], dt.bfloat16, tag="st")
                nc.vector.tensor_tensor(
                    out=st[:],
                    in0=dl_g[:].unsqueeze(2).to_broadcast([128, gch, 128]),
                    in1=iota2[:].unsqueeze(1).to_broadcast([128, gch, 128]),
                    op=mybir.AluOpType.is_equal)

                # msg = hs * ex (in place, broadcast over C)
                nc.vector.tensor_tensor(
                    out=ghs[:, :, 0:D].rearrange("p g (h c) -> p g h c", h=H),
                    in0=ghs[:, :, 0:D].rearrange("p g (h c) -> p g h c", h=H),
                    in1=ex[:].unsqueeze(3).to_broadcast([128, gch, H, C]),
                    op=mybir.AluOpType.mult)

                for b in range(gb0, gb1):
                    pb = min(128, HALF - b * 128)
                    lc0 = ch_off[b] - c0
                    nch = ch_per_block[b]
                    psn = ps2.tile([128, D], dt.float32, tag="psn")
                    psd2 = ps2.tile([128, 8], dt.float32, tag="psd2")
                    for j in range(nch):
                        nc.tensor.matmul(
                            psn[:], lhsT=st[:, lc0 + j, :],
                            rhs=ghs[:, lc0 + j, 0:D],
                            start=(j == 0), stop=(j == nch - 1))
                    for j in range(nch):
                        nc.tensor.matmul(
                            psd2[:], lhsT=st[:, lc0 + j, :],
                            rhs=exb[:, lc0 + j, :],
                            start=(j == 0), stop=(j == nch - 1))
                    rec = p2.tile([128, 8], dt.float32, tag="rec")
                    nc.vector.reciprocal(rec[:pb], psd2[:pb])
                    outb = p2.tile([128, D], dt.float32, tag="outb")
                    nc.vector.scalar_tensor_tensor(
                        out=outb[:pb].rearrange("p (h c) -> p h c", h=H),
                        in0=psn[:pb].rearrange("p (h c) -> p h c", h=H),
                        scalar=1.0,
                        in1=rec[:pb].unsqueeze(2).to_broadcast([pb, H, C]),
                        op0=mybir.AluOpType.mult, op1=mybir.AluOpType.mult)
                    nc.vector.tensor_tensor(out=outb[:pb], in0=outb[:pb],
                                            in1=bias2[:pb], op=mybir.AluOpType.add)
                    aggb = p2.tile([128, D], dt.bfloat16, tag="aggb")
                    nc.scalar.activation(aggb[:pb], outb[:pb],
                                         mybir.ActivationFunctionType.Relu)
                    nc.sync.dma_start(out=agg[b * 128:b * 128 + pb, :], in_=aggb[:pb])

    nc.compile()
    return nc


def _build_launch2():
    from concourse import mybir, bacc
    from concourse.tile import TileContext
    from concourse.masks import make_identity
    dt = mybir.dt
    NS = N // N_CORES            # 2500

    nc = bacc.Bacc("TRN2", target_bir_lowering=False, debug=False,
                   num_devices=N_CORES)

    agg4 = nc.dram_tensor("agg4", [4, NS, D], dt.bfloat16, kind="ExternalInput").ap()
    xn = nc.dram_tensor("xn", [NS, D], dt.bfloat16, kind="ExternalInput").ap()
    xnT = nc.dram_tensor("xnTs", [D, NS], dt.bfloat16, kind="ExternalInput").ap()
    ua_rep = nc.dram_tensor("ua_rep", [128, D], dt.float32, kind="ExternalInput").ap()
    ux_rep = nc.dram_tensor("ux_rep", [128, D], dt.float32, kind="ExternalInput").ap()
    wx = nc.dram_tensor("wx", [D, D], dt.bfloat16, kind="ExternalInput").ap()
    wc = nc.dram_tensor("wc", [D, D], dt.bfloat16, kind="ExternalInput").ap()
    lb_rep = nc.dram_tensor("lb_rep", [128, D], dt.float32, kind="ExternalInput").ap()
    out = nc.dram_tensor("out", [NS, D], dt.float32, kind="ExternalOutput").ap()

    nbf = _ceil(NS, 128)         # 20 (last = 68 rows)

    with TileContext(nc) as tc:
        with tc.tile_pool(name="c", bufs=1) as cp, \
             tc.tile_pool(name="s", bufs=3) as sp, \
             tc.tile_pool(name="ps", bufs=3, space="PSUM") as pp:

            ident = cp.tile([128, 128], dt.bfloat16)
            make_identity(nc, ident)
            ua_s = cp.tile([128, D], dt.float32)
            nc.sync.dma_start(out=ua_s[:], in_=ua_rep[:])
            ux_s = cp.tile([128, D], dt.float32)
            nc.sync.dma_start(out=ux_s[:], in_=ux_rep[:])
            lb_s = cp.tile([128, D], dt.float32)
            nc.sync.dma_start(out=lb_s[:], in_=lb_rep[:])
            wx_s = cp.tile([128, 2, D], dt.bfloat16)
            wc_s = cp.tile([128, 2, D], dt.bfloat16)
            for cc in range(2):
                nc.sync.dma_start(out=wx_s[:, cc, :], in_=wx[cc * 128:(cc + 1) * 128, :])
                nc.sync.dma_start(out=wc_s[:, cc, :], in_=wc[cc * 128:(cc + 1) * 128, :])

            for b in range(nbf):
                P = min(128, NS - b * 128)
                r0 = b * 128
                a_t = sp.tile([128, 4, D], dt.bfloat16, tag="a_t")
                nc.sync.dma_start(out=a_t[:P, :, :],
                                  in_=agg4[:, r0:r0 + P, :].transpose([1, 0, 2]))
                xn_t = sp.tile([128, D], dt.bfloat16, tag="xn_t")
                nc.sync.dma_start(out=xn_t[:P], in_=xn[r0:r0 + P, :])
                xnT_t = sp.tile([128, 2, 128], dt.bfloat16, tag="xnT_t")
                for cc in range(2):
                    nc.sync.dma_start(out=xnT_t[:, cc, :P],
                                      in_=xnT[cc * 128:(cc + 1) * 128, r0:r0 + P])

                sc = sp.tile([128, 8], dt.float32, tag="sc")
                for r in range(4):
                    scr = sp.tile([128, D], dt.float32, tag="scr")
                    nc.vector.scalar_tensor_tensor(
                        out=scr[:P],
                        in0=a_t[:P, r, :], scalar=1.0, in1=ua_s[:P],
                        op0=mybir.AluOpType.mult, op1=mybir.AluOpType.mult,
                        accum_out=sc[:P, r:r + 1])
                scr = sp.tile([128, D], dt.float32, tag="scr")
                nc.vector.scalar_tensor_tensor(
                    out=scr[:P],
                    in0=xn_t[:P], scalar=1.0, in1=ux_s[:P],
                    op0=mybir.AluOpType.mult, op1=mybir.AluOpType.mult,
                    accum_out=sc[:P, 4:5])
                zt = sp.tile([128, 4], dt.float32, tag="zt")
                nc.vector.tensor_tensor(
                    out=zt[:P], in0=sc[:P, 0:4],
                    in1=sc[:P, 4:5].to_broadcast([P, 4]), op=mybir.AluOpType.add)
                nc.vector.scalar_tensor_tensor(
                    out=zt[:P], in0=zt[:P], scalar=float(SEM_SLOPE), in1=zt[:P],
                    op0=mybir.AluOpType.mult, op1=mybir.AluOpType.max)
                nc.scalar.activation(zt[:P], zt[:P], mybir.ActivationFunctionType.Exp)
                ssum = sp.tile([128, 1], dt.float32, tag="ssum")
                nc.vector.tensor_reduce(out=ssum[:P], in_=zt[:P],
                                        axis=mybir.AxisListType.X,
                                        op=mybir.AluOpType.add)
                rs = sp.tile([128, 1], dt.float32, tag="rs")
                nc.vector.reciprocal(rs[:P], ssum[:P])
                w_t = sp.tile([128, 4], dt.float32, tag="w_t")
                nc.vector.tensor_tensor(out=w_t[:P], in0=zt[:P],
                                        in1=rs[:P].to_broadcast([P, 4]),
                                        op=mybir.AluOpType.mult)

                comb_bf = sp.tile([128, D], dt.bfloat16, tag="comb_bf")
                comb = sp.tile([128, D], dt.float32, tag="comb")
                nc.vector.memset(comb[:], 0.0)
                for r in range(4):
                    nc.vector.scalar_tensor_tensor(
                        out=comb[:P], in0=a_t[:P, r, :], scalar=w_t[:P, r:r + 1],
                        in1=comb[:P], op0=mybir.AluOpType.mult,
                        op1=mybir.AluOpType.add)
                nc.vector.tensor_copy(comb_bf[:], comb[:])

                combT = sp.tile([128, 2, 128], dt.bfloat16, tag="combT")
                for t in range(2):
                    pst = pp.tile([128, 128], dt.bfloat16, tag="pst")
                    nc.tensor.transpose(out=pst[:],
                                        in_=comb_bf[:, t * 128:(t + 1) * 128],
                                        identity=ident[:])
                    nc.vector.tensor_copy(combT[:, t, :], pst[:])

                ph = pp.tile([128, D], dt.float32, tag="ph")
                nc.tensor.matmul(ph[:P], lhsT=xnT_t[:, 0, :P], rhs=wx_s[:, 0, :],
                                 start=True, stop=False)
                nc.tensor.matmul(ph[:P], lhsT=xnT_t[:, 1, :P], rhs=wx_s[:, 1, :],
                                 start=False, stop=False)
                nc.tensor.matmul(ph[:P], lhsT=combT[:, 0, :P], rhs=wc_s[:, 0, :],
                                 start=False, stop=False)
                nc.tensor.matmul(ph[:P], lhsT=combT[:, 1, :P], rhs=wc_s[:, 1, :],
                                 start=False, stop=True)

                hb = sp.tile([128, D], dt.float32, tag="hb")
                nc.vector.tensor_tensor(out=hb[:P], in0=ph[:P], in1=lb_s[:P],
                                        op=mybir.AluOpType.add)
                nc.scalar.activation(hb[:P], hb[:P],
                                     mybir.ActivationFunctionType.Relu)
                sq = sp.tile([128, 1], dt.float32, tag="sq")
                sqs = sp.tile([128, D], dt.float32, tag="sqs")
                nc.scalar.activation(sqs[:P],
                                     hb[:P], mybir.ActivationFunctionType.Square,
                                     accum_out=sq[:P])
                nrm = sp.tile([128, 1], dt.float32, tag="nrm")
                nc.scalar.activation(nrm[:P], sq[:P],
                                     mybir.ActivationFunctionType.Sqrt)
                nc.vector.tensor_scalar_max(nrm[:P], nrm[:P], 1e-12)
                rn = sp.tile([128, 1], dt.float32, tag="rn")
                nc.vector.reciprocal(rn[:P], nrm[:P])
                ot = sp.tile([128, D], dt.float32, tag="ot")
                nc.vector.tensor_tensor(out=ot[:P], in0=hb[:P],
                                        in1=rn[:P].to_broadcast([P, D]),
                                        op=mybir.AluOpType.mult)
                nc.sync.dma_start(out=out[r0:r0 + P, :], in_=ot[:P])

    nc.compile()
    return nc


def _prep_edges(edges):
    """Block-sorted padded edge lists per (relation, half)."""
    per_core = []
    for r in range(4):
        src = edges[r, 1].astype(np.int64)
        dst = edges[r, 0].astype(np.int64)
        keep = src != dst
        src = np.concatenate([src[keep], np.arange(N, dtype=np.int64)])
        dst = np.concatenate([dst[keep], np.arange(N, dtype=np.int64)])
        for h in (0, 1):
            m = (dst >= h * HALF) & (dst < (h + 1) * HALF)
            s, d = src[m], dst[m] - h * HALF
            order = np.argsort(d // 128, kind="stable")
            per_core.append((s[order], d[order]))

    nblk = _ceil(HALF, 128)
    counts = np.zeros((8, nblk), np.int64)
    for ci, (s, d) in enumerate(per_core):
        counts[ci] = np.bincount(d // 128, minlength=nblk)
    ch_per_block = [max(1, int(x)) for x in
                    np.ceil(counts.max(0) / 128).astype(np.int64)]
    tot_ch = sum(ch_per_block)

    cores = []
    for ci, (s, d) in enumerate(per_core):
        gsrc = np.full(tot_ch * 128, N, np.int64)       # dummy -> hs_t row N
        gdst = np.full(tot_ch * 128, HALF, np.int64)    # dummy -> ald_t row HALF
        dloc = np.zeros(tot_ch * 128, np.int64)
        off = 0
        epos = 0
        for b in range(nblk):
            cnt = counts[ci, b]
            gsrc[off:off + cnt] = s[epos:epos + cnt]
            gdst[off:off + cnt] = d[epos:epos + cnt]
            dloc[off:off + cnt] = d[epos:epos + cnt] - b * 128
            epos += cnt
            off += ch_per_block[b] * 128
        cores.append((gsrc, gdst, dloc))
    return ch_per_block, tot_ch, cores


def _wrap16(a):
    w = a.reshape(-1, 16).T
    return np.tile(w, (8, 1)).astype(np.int16)


def kernel(x_src, x_node, edges, ew, W_src, W_dst, att_src, att_dst,
           bias, u, lin_W, lin_b):
    from concourse.bass_utils import run_bass_kernel_spmd

    x_src = np.asarray(x_src, np.float32)
    x_node = np.asarray(x_node, np.float32)
    edges = np.asarray(edges)
    W_src = np.asarray(W_src, np.float32)
    W_dst = np.asarray(W_dst, np.float32)
    att_src = np.asarray(att_src, np.float32)
    att_dst = np.asarray(att_dst, np.float32)
    bias = np.asarray(bias, np.float32)
    u = np.asarray(u, np.float32).reshape(2 * D)
    lin_W = np.asarray(lin_W, np.float32)
    lin_b = np.asarray(lin_b, np.float32)

    ch_per_block, tot_ch, cores_idx = _prep_edges(edges)

    key = ("l1", tuple(ch_per_block))
    if key not in _cache:
        nblk = len(ch_per_block)
        splits = []
        b = 0
        while b < nblk:
            splits.append((b, min(nblk, b + 2)))
            b += 2
        _cache[key] = _build_launch1(ch_per_block, splits, tot_ch)
    nc1 = _cache[key]
    if "l2" not in _cache:
        _cache["l2"] = _build_launch2()
    nc2 = _cache["l2"]

    # fold attention vectors into the weight matrices
    Wr = W_src.reshape(4, D, H, C)
    ws_fold = np.einsum("rdhc,rhc->rdh", Wr, att_src)
    Wd = W_dst.reshape(4, D, H, C)
    wd_fold = np.einsum("rdhc,rhc->rdh", Wd, att_dst)

    in_maps1 = []
    for c in range(N_CORES):
        r, h = c // 2, c % 2
        gs, gd, dl = cores_idx[r * 2 + h]
        in_maps1.append(dict(
            xsT=np.ascontiguousarray(x_src[r].T).astype(BF16),
            xnT=np.ascontiguousarray(x_node[h * HALF:(h + 1) * HALF].T).astype(BF16),
            wsp=np.concatenate([W_src[r], ws_fold[r]], axis=1).astype(BF16),
            wdf=wd_fold[r].astype(BF16),
            bias_rep=np.tile(bias[r][None, :], (128, 1)).astype(np.float32),
            gsrc=_wrap16(gs),
            gdst=_wrap16(gd),
            dstloc=dl.reshape(tot_ch, 128).T.astype(np.float32).copy(),
        ))

    import time as _time
    trace = bool(int(os.environ.get("KERNEL_TRACE", "0")))
    t0 = _time.time()
    r1 = run_bass_kernel_spmd(nc1, in_maps1, list(range(N_CORES)), trace=trace)
    LAST_RUN_INFO["l1_wall"] = _time.time() - t0
    LAST_RUN_INFO["l1_exec_ns"] = r1.exec_time_ns
    LAST_RUN_INFO["r1"] = r1
    aggs = [np.asarray(r1.results[c]["agg"]) for c in range(N_CORES)]

    NS = N // N_CORES
    ua, uxv = u[:D], u[D:]
    in_maps2 = []
    for c2 in range(N_CORES):
        h = c2 // 4
        lo = c2 * NS - h * HALF
        agg4 = np.stack([aggs[r * 2 + h][lo:lo + NS] for r in range(4)])
        xn_sl = x_node[c2 * NS:(c2 + 1) * NS]
        in_maps2.append(dict(
            agg4=agg4,
            xn=xn_sl.astype(BF16),
            xnTs=np.ascontiguousarray(xn_sl.T).astype(BF16),
            ua_rep=np.tile(ua[None, :], (128, 1)).astype(np.float32),
            ux_rep=np.tile(uxv[None, :], (128, 1)).astype(np.float32),
            wx=lin_W[:D].astype(BF16),
            wc=lin_W[D:].astype(BF16),
            lb_rep=np.tile(lin_b[None, :], (128, 1)).astype(np.float32),
        ))

    t0 = _time.time()
    r2 = run_bass_kernel_spmd(nc2, in_maps2, list(range(N_CORES)), trace=trace)
    LAST_RUN_INFO["l2_wall"] = _time.time() - t0
    LAST_RUN_INFO["l2_exec_ns"] = r2.exec_time_ns
    LAST_RUN_INFO["r2"] = r2
    out = np.concatenate([np.asarray(r2.results[c]["out"]) for c in range(N_CORES)])
    return out.astype(np.float32)


if __name__ == "__main__":
    # quick self-test in CoreSim for core 0 is done via test_sim.py
    pass


# revision 16
# speedup vs baseline: 3525.6257x; 3525.6257x over previous
"""nn_HAN_Agg on 8 Trainium2 NeuronCores via Bass/Tile.

Sharding: core c = (relation r = c//2, dst-half = c%2).
Launch 1: per-relation GATConv aggregation. hs_pack = x_src[r] @ [W_src |
fold(att_src)] written to a DRAM table with 768B rows [hs(256) | al_s(8) |
pad]; al_d likewise in a 256B-row table.  Per 128-dst block, edges (sorted
by dst block, padded to multiples of 128, uniform chunk counts across
cores for SPMD) are processed as: dma_gather of hs+al_s rows and al_d
rows, ex = exp(leaky(al_s+al_d)), one-hot S^T = (dstloc == iota), then
TensorE matmuls accumulate num = S^T.T @ (ex*hs) and den = S^T.T @ ex in
PSUM.  agg = relu(num/den + bias).
Launch 2 (after a host reshuffle): semantic attention + linear head + L2
normalize, 2500 nodes per core.
"""
import os, sys
sys.path.insert(0, "/opt/trn_rl_repo")
import numpy as np
import ml_dtypes

BF16 = ml_dtypes.bfloat16

N, D, H, E = 20000, 256, 8, 320000
C = D // H
GAT_SLOPE = 0.2
SEM_SLOPE = 0.01
N_CORES = 8
HALF = N // 2                    # dst nodes per core (10000)
ROWW = 384                       # packed hs row: [hs 256 | al_s 8 | pad] bf16 = 768B

_cache = {}
LAST_RUN_INFO = {}


def _install_ntff_hook():
    """Provide antenv.axon_hooks.get_axon_ntff_profile_hook via ctypes
    against /opt/axon/libaxon_pjrt.so (the image's antenv lacks it)."""
    import types, contextlib, ctypes
    import antenv
    if hasattr(antenv, "axon_hooks"):
        return
    so_path = "/opt/axon/libaxon_pjrt.so"
    if not os.path.exists(so_path):
        return
    lib = ctypes.CDLL(so_path)
    if not hasattr(lib, "axon_start_nrt_profile"):
        return
    lib.axon_start_nrt_profile.argtypes = [ctypes.POINTER(ctypes.c_int64), ctypes.c_size_t]
    lib.axon_start_nrt_profile.restype = ctypes.c_int64
    lib.axon_stop_nrt_profile.argtypes = [ctypes.c_char_p]
    lib.axon_stop_nrt_profile.restype = ctypes.c_int64

    @contextlib.contextmanager
    def _hook(output_dir, device_ids):
        import jax
        jax.devices()
        if device_ids:
            ids = (ctypes.c_int64 * len(device_ids))(*device_ids)
            rc = lib.axon_start_nrt_profile(ids, len(device_ids))
        else:
            rc = lib.axon_start_nrt_profile(None, 0)
        if rc != 0:
            raise RuntimeError(f"axon_start_nrt_profile rc={rc}")
        try:
            yield
        finally:
            n = lib.axon_stop_nrt_profile(str(output_dir).encode())
            print(f"ntff profile: {n} file(s) -> {output_dir}", file=sys.stderr)

    mod = types.ModuleType("antenv.axon_hooks")
    mod.get_axon_ntff_profile_hook = lambda: _hook
    mod.set_axon_ntff_profile_hook = lambda h: None
    sys.modules["antenv.axon_hooks"] = mod
    antenv.axon_hooks = mod



def _ceil(a, b):
    return (a + b - 1) // b


def _build_launch1(ch_per_block, group_splits, tot_ch):
    from concourse import mybir, bacc
    from concourse.tile import TileContext
    dt = mybir.dt
    nblk = len(ch_per_block)
    n_e16 = tot_ch * 8           # idx cols (tot_ch*128/16)

    nc = bacc.Bacc("TRN2", target_bir_lowering=False, debug=False,
                   num_devices=N_CORES)

    xsT = nc.dram_tensor("xsT", [D, N], dt.bfloat16, kind="ExternalInput").ap()
    xnT = nc.dram_tensor("xnT", [D, HALF], dt.bfloat16, kind="ExternalInput").ap()
    wsp = nc.dram_tensor("wsp", [D, D + 8], dt.bfloat16, kind="ExternalInput").ap()
    wdf = nc.dram_tensor("wdf", [D, 8], dt.bfloat16, kind="ExternalInput").ap()
    bias_rep = nc.dram_tensor("bias_rep", [128, D], dt.float32, kind="ExternalInput").ap()
    gsrc = nc.dram_tensor("gsrc", [128, n_e16], dt.int16, kind="ExternalInput").ap()
    gdst = nc.dram_tensor("gdst", [128, n_e16], dt.int16, kind="ExternalInput").ap()
    dstloc = nc.dram_tensor("dstloc", [128, tot_ch], dt.float32, kind="ExternalInput").ap()

    hs_t = nc.dram_tensor("hs_t", [N + 8, ROWW], dt.bfloat16)
    ald_t = nc.dram_tensor("ald_t", [HALF + 8, 128], dt.bfloat16)

    agg = nc.dram_tensor("agg", [HALF, D], dt.bfloat16, kind="ExternalOutput").ap()

    nblk_a = _ceil(N, 128)       # 157 (last = 32 rows)

    with TileContext(nc) as tc:
        with tc.tile_pool(name="c1", bufs=1) as cp, \
             tc.tile_pool(name="p1", bufs=3) as p1, \
             tc.tile_pool(name="ps1", bufs=3, space="PSUM") as ps1:

            wsp_s = cp.tile([128, 2, D + 8], dt.bfloat16)
            wdf_s = cp.tile([128, 2, 8], dt.bfloat16)
            for cc in range(2):
                nc.sync.dma_start(out=wsp_s[:, cc, :], in_=wsp[cc * 128:(cc + 1) * 128, :])
                nc.sync.dma_start(out=wdf_s[:, cc, :], in_=wdf[cc * 128:(cc + 1) * 128, :])

            # dummy rows: hs_t[N] = [0.. | -1e9 x8 | 0..], ald_t[HALF] = 0
            drow = cp.tile([1, ROWW], dt.bfloat16)
            nc.vector.memset(drow[:], 0.0)
            nc.vector.memset(drow[:, D:D + 8], -1e9)
            nc.sync.dma_start(out=hs_t[N, :].unsqueeze(0), in_=drow[:])
            zrow = cp.tile([1, 128], dt.bfloat16)
            nc.vector.memset(zrow[:], 0.0)
            nc.sync.dma_start(out=ald_t[HALF, :].unsqueeze(0), in_=zrow[:])

            # phase 1a: hs_pack for all N rows
            for b4 in range(0, nblk_a, 4):
                hi = min(N, b4 * 128 + 512)
                w = hi - b4 * 128
                lhs = p1.tile([128, 2, 512], dt.bfloat16, tag="lhs")
                for cc in range(2):
                    nc.sync.dma_start(out=lhs[:, cc, :w],
                                      in_=xsT[cc * 128:(cc + 1) * 128, b4 * 128:hi])
                for j in range(_ceil(w, 128)):
                    blk = b4 + j
                    pb = min(128, N - blk * 128)
                    ps = ps1.tile([128, D + 8], dt.float32, tag="psa")
                    for cc in range(2):
                        nc.tensor.matmul(
                            ps[:pb],
                            lhsT=lhs[:, cc, j * 128:j * 128 + pb],
                            rhs=wsp_s[:, cc, :],
                            start=(cc == 0), stop=(cc == 1))
                    hsb = p1.tile([128, D + 8], dt.bfloat16, tag="hsb")
                    nc.scalar.activation(hsb[:pb], ps[:pb],
                                         mybir.ActivationFunctionType.Copy)
                    nc.sync.dma_start(
                        out=hs_t[blk * 128:blk * 128 + pb, 0:D + 8], in_=hsb[:pb])

            # phase 1b: al_d for owned half
            for b4 in range(0, nblk, 4):
                hi = min(HALF, b4 * 128 + 512)
                w = hi - b4 * 128
                lhd = p1.tile([128, 2, 512], dt.bfloat16, tag="lhd")
                for cc in range(2):
                    nc.sync.dma_start(out=lhd[:, cc, :w],
                                      in_=xnT[cc * 128:(cc + 1) * 128, b4 * 128:hi])
                for j in range(_ceil(w, 128)):
                    blk = b4 + j
                    pb = min(128, HALF - blk * 128)
                    psd = ps1.tile([128, 8], dt.float32, tag="psd")
                    for cc in range(2):
                        nc.tensor.matmul(
                            psd[:pb],
                            lhsT=lhd[:, cc, j * 128:j * 128 + pb],
                            rhs=wdf_s[:, cc, :],
                            start=(cc == 0), stop=(cc == 1))
                    aldb = p1.tile([128, 8], dt.bfloat16, tag="aldb")
                    nc.vector.tensor_copy(aldb[:pb], psd[:pb])
                    nc.sync.dma_start(
                        out=ald_t[blk * 128:blk * 128 + pb, 0:8], in_=aldb[:pb])

        # phase 2: edge aggregation
        with tc.tile_pool(name="c2", bufs=1) as cp2, \
             tc.tile_pool(name="p2", bufs=2) as p2, \
             tc.tile_pool(name="ps2", bufs=3, space="PSUM") as ps2:

            iota2 = cp2.tile([128, 128], dt.float32)
            nc.gpsimd.iota(iota2[:], pattern=[[1, 128]], channel_multiplier=0,
                           allow_small_or_imprecise_dtypes=True)
            bias2 = cp2.tile([128, D], dt.float32)
            nc.sync.dma_start(out=bias2[:], in_=bias_rep[:])

            ch_off = [0]
            for b in range(nblk):
                ch_off.append(ch_off[-1] + ch_per_block[b])

            _maxg = int(os.environ.get("L1_MAX_GROUPS", "9999"))
            for (gb0, gb1) in group_splits[:_maxg]:
                c0, c1 = ch_off[gb0], ch_off[gb1]
                gch = c1 - c0
                ne = gch * 128

                gsrc_g = p2.tile([128, gch * 8], dt.int16, tag="gsrc_g")
                nc.sync.dma_start(out=gsrc_g[:], in_=gsrc[:, c0 * 8:c1 * 8])
                gdst_g = p2.tile([128, gch * 8], dt.int16, tag="gdst_g")
                nc.sync.dma_start(out=gdst_g[:], in_=gdst[:, c0 * 8:c1 * 8])
                dl_g = p2.tile([128, gch], dt.float32, tag="dl_g")
                nc.sync.dma_start(out=dl_g[:], in_=dstloc[:, c0:c1])

                _og = os.environ.get("L1_ONLY_G", "")
                ghs = p2.tile([128, gch, ROWW], dt.bfloat16, tag="ghs")
                if _og in ("", "hs"):
                    nc.gpsimd.dma_gather(
                        out_ap=ghs[:], in_ap=hs_t[:], idxs_ap=gsrc_g[:],
                        num_idxs=ne, num_idxs_reg=ne, elem_size=ROWW, queue_num=0,
                        single_packet=False)
                gad = p2.tile([128, gch, 128], dt.bfloat16, tag="gad")
                if _og in ("", "ad"):
                    nc.gpsimd.dma_gather(
                        out_ap=gad[:], in_ap=ald_t[:], idxs_ap=gdst_g[:],
                        num_idxs=ne, num_idxs_reg=ne, elem_size=128, queue_num=0,
                        single_packet=False)

                if os.environ.get("L1_GATHER_ONLY"):
                    continue
                # ex = exp(leaky(al_s + al_d))
                ex = p2.tile([128, gch, 8], dt.float32, tag="ex")
                nc.vector.tensor_tensor(out=ex[:], in0=ghs[:, :, D:D + 8],
                                        in1=gad[:, :, 0:8], op=mybir.AluOpType.add)
                nc.vector.scalar_tensor_tensor(
                    out=ex[:], in0=ex[:], scalar=float(GAT_SLOPE), in1=ex[:],
                    op0=mybir.AluOpType.mult, op1=mybir.AluOpType.max)
                nc.scalar.activation(ex[:], ex[:], mybir.ActivationFunctionType.Exp)
                exb = p2.tile([128, gch, 8], dt.bfloat16, tag="exb")
                nc.vector.tensor_copy(exb[:], ex[:])

                # S^T one-hot (dstloc == iota)
                st = p2.tile([128, gch, 128], dt.bfloat16, tag="st")
                nc.vector.tensor_tensor(
                    out=st[:],
                    in0=dl_g[:].unsqueeze(2).to_broadcast([128, gch, 128]),
                    in1=iota2[:].unsqueeze(1).to_broadcast([128, gch, 128]),
                    op=mybir.AluOpType.is_equal)

                # msg = hs * ex (in place, broadcast over C)
                nc.vector.tensor_tensor(
                    out=ghs[:, :, 0:D].rearrange("p g (h c) -> p g h c", h=H),
                    in0=ghs[:, :, 0:D].rearrange("p g (h c) -> p g h c", h=H),
                    in1=ex[:].unsqueeze(3).to_broadcast([128, gch, H, C]),
                    op=mybir.AluOpType.mult)

                for b in range(gb0, gb1):
                    pb = min(128, HALF - b * 128)
                    lc0 = ch_off[b] - c0
                    nch = ch_per_block[b]
                    psn = ps2.tile([128, D], dt.float32, tag="psn")
                    psd2 = ps2.tile([128, 8], dt.float32, tag="psd2")
                    for j in range(nch):
                        nc.tensor.matmul(
                            psn[:], lhsT=st[:, lc0 + j, :],
                            rhs=ghs[:, lc0 + j, 0:D],
                            start=(j == 0), stop=(j == nch - 1))
                    for j in range(nch):
                        nc.tensor.matmul(
                            psd2[:], lhsT=st[:, lc0 + j, :],
                            rhs=exb[:, lc0 + j, :],
                            start=(j == 0), stop=(j == nch - 1))
                    rec = p2.tile([128, 8], dt.float32, tag="rec")
                    nc.vector.reciprocal(rec[:pb], psd2[:pb])
                    outb = p2.tile([128, D], dt.float32, tag="outb")
                    nc.vector.scalar_tensor_tensor(
                        out=outb[:pb].rearrange("p (h c) -> p h c", h=H),
                        in0=psn[:pb].rearrange("p (h c) -> p h c", h=H),
                        scalar=1.0,
                        in1=rec[:pb].unsqueeze(2).to_broadcast([pb, H, C]),
                        op0=mybir.AluOpType.mult, op1=mybir.AluOpType.mult)
                    nc.vector.tensor_tensor(out=outb[:pb], in0=outb[:pb],
                                            in1=bias2[:pb], op=mybir.AluOpType.add)
                    aggb = p2.tile([128, D], dt.bfloat16, tag="aggb")
                    nc.scalar.activation(aggb[:pb], outb[:pb],
                                         mybir.ActivationFunctionType.Relu)
                    nc.sync.dma_start(out=agg[b * 128:b * 128 + pb, :], in_=aggb[:pb])

    nc.compile()
    return nc


def _build_launch2():
    from concourse import mybir, bacc
    from concourse.tile import TileContext
    from concourse.masks import make_identity
    dt = mybir.dt
    NS = N // N_CORES            # 2500

    nc = bacc.Bacc("TRN2", target_bir_lowering=False, debug=False,
                   num_devices=N_CORES)

    agg4 = nc.dram_tensor("agg4", [4, NS, D], dt.bfloat16, kind="ExternalInput").ap()
    xn = nc.dram_tensor("xn", [NS, D], dt.bfloat16, kind="ExternalInput").ap()
    xnT = nc.dram_tensor("xnTs", [D, NS], dt.bfloat16, kind="ExternalInput").ap()
    ua_rep = nc.dram_tensor("ua_rep", [128, D], dt.float32, kind="ExternalInput").ap()
    ux_rep = nc.dram_tensor("ux_rep", [128, D], dt.float32, kind="ExternalInput").ap()
    wx = nc.dram_tensor("wx", [D, D], dt.bfloat16, kind="ExternalInput").ap()
    wc = nc.dram_tensor("wc", [D, D], dt.bfloat16, kind="ExternalInput").ap()
    lb_rep = nc.dram_tensor("lb_rep", [128, D], dt.float32, kind="ExternalInput").ap()
    out = nc.dram_tensor("out", [NS, D], dt.float32, kind="ExternalOutput").ap()

    nbf = _ceil(NS, 128)         # 20 (last = 68 rows)

    with TileContext(nc) as tc:
        with tc.tile_pool(name="c", bufs=1) as cp, \
             tc.tile_pool(name="s", bufs=3) as sp, \
             tc.tile_pool(name="ps", bufs=3, space="PSUM") as pp:

            ident = cp.tile([128, 128], dt.bfloat16)
            make_identity(nc, ident)
            ua_s = cp.tile([128, D], dt.float32)
            nc.sync.dma_start(out=ua_s[:], in_=ua_rep[:])
            ux_s = cp.tile([128, D], dt.float32)
            nc.sync.dma_start(out=ux_s[:], in_=ux_rep[:])
            lb_s = cp.tile([128, D], dt.float32)
            nc.sync.dma_start(out=lb_s[:], in_=lb_rep[:])
            wx_s = cp.tile([128, 2, D], dt.bfloat16)
            wc_s = cp.tile([128, 2, D], dt.bfloat16)
            for cc in range(2):
                nc.sync.dma_start(out=wx_s[:, cc, :], in_=wx[cc * 128:(cc + 1) * 128, :])
                nc.sync.dma_start(out=wc_s[:, cc, :], in_=wc[cc * 128:(cc + 1) * 128, :])

            for b in range(nbf):
                P = min(128, NS - b * 128)
                r0 = b * 128
                a_t = sp.tile([128, 4, D], dt.bfloat16, tag="a_t")
                nc.sync.dma_start(out=a_t[:P, :, :],
                                  in_=agg4[:, r0:r0 + P, :].transpose([1, 0, 2]))
                xn_t = sp.tile([128, D], dt.bfloat16, tag="xn_t")
                nc.sync.dma_start(out=xn_t[:P], in_=xn[r0:r0 + P, :])
                xnT_t = sp.tile([128, 2, 128], dt.bfloat16, tag="xnT_t")
                for cc in range(2):
                    nc.sync.dma_start(out=xnT_t[:, cc, :P],
                                      in_=xnT[cc * 128:(cc + 1) * 128, r0:r0 + P])

                sc = sp.tile([128, 8], dt.float32, tag="sc")
                for r in range(4):
                    scr = sp.tile([128, D], dt.float32, tag="scr")
                    nc.vector.scalar_tensor_tensor(
                        out=scr[:P],
                        in0=a_t[:P, r, :], scalar=1.0, in1=ua_s[:P],
                        op0=mybir.AluOpType.mult, op1=mybir.AluOpType.mult,
                        accum_out=sc[:P, r:r + 1])
                scr = sp.tile([128, D], dt.float32, tag="scr")
                nc.vector.scalar_tensor_tensor(
                    out=scr[:P],
                    in0=xn_t[:P], scalar=1.0, in1=ux_s[:P],
                    op0=mybir.AluOpType.mult, op1=mybir.AluOpType.mult,
                    accum_out=sc[:P, 4:5])
                zt = sp.tile([128, 4], dt.float32, tag="zt")
                nc.vector.tensor_tensor(
                    out=zt[:P], in0=sc[:P, 0:4],
                    in1=sc[:P, 4:5].to_broadcast([P, 4]), op=mybir.AluOpType.add)
                nc.vector.scalar_tensor_tensor(
                    out=zt[:P], in0=zt[:P], scalar=float(SEM_SLOPE), in1=zt[:P],
                    op0=mybir.AluOpType.mult, op1=mybir.AluOpType.max)
                nc.scalar.activation(zt[:P], zt[:P], mybir.ActivationFunctionType.Exp)
                ssum = sp.tile([128, 1], dt.float32, tag="ssum")
                nc.vector.tensor_reduce(out=ssum[:P], in_=zt[:P],
                                        axis=mybir.AxisListType.X,
                                        op=mybir.AluOpType.add)
                rs = sp.tile([128, 1], dt.float32, tag="rs")
                nc.vector.reciprocal(rs[:P], ssum[:P])
                w_t = sp.tile([128, 4], dt.float32, tag="w_t")
                nc.vector.tensor_tensor(out=w_t[:P], in0=zt[:P],
                                        in1=rs[:P].to_broadcast([P, 4]),
                                        op=mybir.AluOpType.mult)

                comb_bf = sp.tile([128, D], dt.bfloat16, tag="comb_bf")
                comb = sp.tile([128, D], dt.float32, tag="comb")
                nc.vector.memset(comb[:], 0.0)
                for r in range(4):
                    nc.vector.scalar_tensor_tensor(
                        out=comb[:P], in0=a_t[:P, r, :], scalar=w_t[:P, r:r + 1],
                        in1=comb[:P], op0=mybir.AluOpType.mult,
                        op1=mybir.AluOpType.add)
                nc.vector.tensor_copy(comb_bf[:], comb[:])

                combT = sp.tile([128, 2, 128], dt.bfloat16, tag="combT")
                for t in range(2):
                    pst = pp.tile([128, 128], dt.bfloat16, tag="pst")
                    nc.tensor.transpose(out=pst[:],
                                        in_=comb_bf[:, t * 128:(t + 1) * 128],
                                        identity=ident[:])
                    nc.vector.tensor_copy(combT[:, t, :], pst[:])

                ph = pp.tile([128, D], dt.float32, tag="ph")
                nc.tensor.matmul(ph[:P], lhsT=xnT_t[:, 0, :P], rhs=wx_s[:, 0, :],
                                 start=True, stop=False)
                nc.tensor.matmul(ph[:P], lhsT=xnT_t[:, 1, :P], rhs=wx_s[:, 1, :],
                                 start=False, stop=False)
                nc.tensor.matmul(ph[:P], lhsT=combT[:, 0, :P], rhs=wc_s[:, 0, :],
                                 start=False, stop=False)
                nc.tensor.matmul(ph[:P], lhsT=combT[:, 1, :P], rhs=wc_s[:, 1, :],
                                 start=False, stop=True)

                hb = sp.tile([128, D], dt.float32, tag="hb")
                nc.vector.tensor_tensor(out=hb[:P], in0=ph[:P], in1=lb_s[:P],
                                        op=mybir.AluOpType.add)
                nc.scalar.activation(hb[:P], hb[:P],
                                     mybir.ActivationFunctionType.Relu)
                sq = sp.tile([128, 1], dt.float32, tag="sq")
                sqs = sp.tile([128, D], dt.float32, tag="sqs")
                nc.scalar.activation(sqs[:P],
                                     hb[:P], mybir.ActivationFunctionType.Square,
                                     accum_out=sq[:P])
                nrm = sp.tile([128, 1], dt.float32, tag="nrm")
                nc.scalar.activation(nrm[:P], sq[:P],
                                     mybir.ActivationFunctionType.Sqrt)
                nc.vector.tensor_scalar_max(nrm[:P], nrm[:P], 1e-12)
                rn = sp.tile([128, 1], dt.float32, tag="rn")
                nc.vector.reciprocal(rn[:P], nrm[:P])
                ot = sp.tile([128, D], dt.float32, tag="ot")
                nc.vector.tensor_tensor(out=ot[:P], in0=hb[:P],
                                        in1=rn[:P].to_broadcast([P, D]),
                                        op=mybir.AluOpType.mult)
                nc.sync.dma_start(out=out[r0:r0 + P, :], in_=ot[:P])

    nc.compile()
    return nc


def _prep_edges(edges):
    """Block-sorted padded edge lists per (relation, half)."""
    per_core = []
    for r in range(4):
        src = edges[r, 1].astype(np.int64)
        dst = edges[r, 0].astype(np.int64)
        keep = src != dst
        src = np.concatenate([src[keep], np.arange(N, dtype=np.int64)])
        dst = np.concatenate([dst[keep], np.arange(N, dtype=np.int64)])
        for h in (0, 1):
            m = (dst >= h * HALF) & (dst < (h + 1) * HALF)
            s, d = src[m], dst[m] - h * HALF
            order = np.argsort(d // 128, kind="stable")
            per_core.append((s[order], d[order]))

    nblk = _ceil(HALF, 128)
    counts = np.zeros((8, nblk), np.int64)
    for ci, (s, d) in enumerate(per_core):
        counts[ci] = np.bincount(d // 128, minlength=nblk)
    ch_per_block = [max(1, int(x)) for x in
                    np.ceil(counts.max(0) / 128).astype(np.int64)]
    tot_ch = sum(ch_per_block)

    cores = []
    for ci, (s, d) in enumerate(per_core):
        gsrc = np.full(tot_ch * 128, N, np.int64)       # dummy -> hs_t row N
        gdst = np.full(tot_ch * 128, HALF, np.int64)    # dummy -> ald_t row HALF
        dloc = np.zeros(tot_ch * 128, np.int64)
        off = 0
        epos = 0
        for b in range(nblk):
            cnt = counts[ci, b]
            gsrc[off:off + cnt] = s[epos:epos + cnt]
            gdst[off:off + cnt] = d[epos:epos + cnt]
            dloc[off:off + cnt] = d[epos:epos + cnt] - b * 128
            epos += cnt
            off += ch_per_block[b] * 128
        cores.append((gsrc, gdst, dloc))
    return ch_per_block, tot_ch, cores


def _wrap16(a):
    w = a.reshape(-1, 16).T
    return np.tile(w, (8, 1)).astype(np.int16)


def kernel(x_src, x_node, edges, ew, W_src, W_dst, att_src, att_dst,
           bias, u, lin_W, lin_b):
    from concourse.bass_utils import run_bass_kernel_spmd

    x_src = np.asarray(x_src, np.float32)
    x_node = np.asarray(x_node, np.float32)
    edges = np.asarray(edges)
    W_src = np.asarray(W_src, np.float32)
    W_dst = np.asarray(W_dst, np.float32)
    att_src = np.asarray(att_src, np.float32)
    att_dst = np.asarray(att_dst, np.float32)
    bias = np.asarray(bias, np.float32)
    u = np.asarray(u, np.float32).reshape(2 * D)
    lin_W = np.asarray(lin_W, np.float32)
    lin_b = np.asarray(lin_b, np.float32)

    ch_per_block, tot_ch, cores_idx = _prep_edges(edges)

    key = ("l1", tuple(ch_per_block))
    if key not in _cache:
        nblk = len(ch_per_block)
        splits = []
        b = 0
        while b < nblk:
            splits.append((b, min(nblk, b + 2)))
            b += 2
        _cache[key] = _build_launch1(ch_per_block, splits, tot_ch)
    nc1 = _cache[key]
    if "l2" not in _cache:
        _cache["l2"] = _build_launch2()
    nc2 = _cache["l2"]

    # fold attention vectors into the weight matrices
    Wr = W_src.reshape(4, D, H, C)
    ws_fold = np.einsum("rdhc,rhc->rdh", Wr, att_src)
    Wd = W_dst.reshape(4, D, H, C)
    wd_fold = np.einsum("rdhc,rhc->rdh", Wd, att_dst)

    in_maps1 = []
    for c in range(N_CORES):
        r, h = c // 2, c % 2
        gs, gd, dl = cores_idx[r * 2 + h]
        in_maps1.append(dict(
            xsT=np.ascontiguousarray(x_src[r].T).astype(BF16),
            xnT=np.ascontiguousarray(x_node[h * HALF:(h + 1) * HALF].T).astype(BF16),
            wsp=np.concatenate([W_src[r], ws_fold[r]], axis=1).astype(BF16),
            wdf=wd_fold[r].astype(BF16),
            bias_rep=np.tile(bias[r][None, :], (128, 1)).astype(np.float32),
            gsrc=_wrap16(gs),
            gdst=_wrap16(gd),
            dstloc=dl.reshape(tot_ch, 128).T.astype(np.float32).copy(),
        ))

    import time as _time
    trace = bool(int(os.environ.get("KERNEL_TRACE", "0")))
    if trace:
        _install_ntff_hook()
    t0 = _time.time()
    r1 = run_bass_kernel_spmd(nc1, in_maps1, list(range(N_CORES)), trace=trace)
    LAST_RUN_INFO["l1_wall"] = _time.time() - t0
    LAST_RUN_INFO["l1_exec_ns"] = r1.exec_time_ns
    LAST_RUN_INFO["r1"] = r1
    aggs = [np.asarray(r1.results[c]["agg"]) for c in range(N_CORES)]

    NS = N // N_CORES
    ua, uxv = u[:D], u[D:]
    in_maps2 = []
    for c2 in range(N_CORES):
        h = c2 // 4
        lo = c2 * NS - h * HALF
        agg4 = np.stack([aggs[r * 2 + h][lo:lo + NS] for r in range(4)])
        xn_sl = x_node[c2 * NS:(c2 + 1) * NS]
        in_maps2.append(dict(
            agg4=agg4,
            xn=xn_sl.astype(BF16),
            xnTs=np.ascontiguousarray(xn_sl.T).astype(BF16),
            ua_rep=np.tile(ua[None, :], (128, 1)).astype(np.float32),
            ux_rep=np.tile(uxv[None, :], (128, 1)).astype(np.float32),
            wx=lin_W[:D].astype(BF16),
            wc=lin_W[D:].astype(BF16),
            lb_rep=np.tile(lin_b[None, :], (128, 1)).astype(np.float32),
        ))

    t0 = _time.time()
    r2 = run_bass_kernel_spmd(nc2, in_maps2, list(range(N_CORES)), trace=trace)
    LAST_RUN_INFO["l2_wall"] = _time.time() - t0
    LAST_RUN_INFO["l2_exec_ns"] = r2.exec_time_ns
    LAST_RUN_INFO["r2"] = r2
    out = np.concatenate([np.asarray(r2.results[c]["out"]) for c in range(N_CORES)])
    return out.astype(np.float32)


if __name__ == "__main__":
    # quick self-test in CoreSim for core 0 is done via test_sim.py
    pass
